# revision 13
# baseline (speedup 1.0000x reference)
"""Trainium2 kernel for nn_Circuit_88871463288913.

Circuit (d=2, n=11 wires, D=2048, B=32):
  psi -> H on every wire -> CNOT ladder -> CRX ladder.

All gates are fixed up to 10 CRX angles, so the full D x D unitary is
composed on the host in O(n * D^2) elementwise work:
  - H^{otimes 11}: Walsh matrix, W[r,c] = (-1)^popcount(r&c) / sqrt(D)
  - CNOT ladder: a row permutation (prefix-XOR over bits)
  - each CRX: psi'[i] = cos(a_i) psi[i] - 1j sin(a_i) psi[flip_t(i)],
    a_i = control_bit(i) * theta/2  (a is invariant under flip_t)

The device then applies the single complex matmul U @ psi, row-sharded
across 8 NeuronCores: core c computes output rows [256c, 256c+256) from
the full (replicated) input state.
"""

import numpy as np

_D = 2048
_N = 11
_B = 32
_NC = 8
_ROWS = _D // _NC  # 256 output rows per core

_LAST = {}  # stash of the most recent BassKernelResults (for test harness)
_PROG = {}  # cached Bass program


def _build_unitary(angles):
    """Compose the full circuit unitary. Returns (Ur, Ui) float32 (D, D)."""
    n, D = _N, _D
    idx = np.arange(D)

    # Walsh-Hadamard: sign = (-1)^popcount(r & c)
    v = idx[:, None] & idx[None, :]
    v = v ^ (v >> 8)
    v = v ^ (v >> 4)
    v = v ^ (v >> 2)
    v = v ^ (v >> 1)
    Mr = ((1.0 - 2.0 * (v & 1)) / np.sqrt(D)).astype(np.float64)
    Mi = np.zeros_like(Mr)

    cols = idx
    # CNOT ladder: M' = P_q @ M, i.e. M'[rows] = M[cols]
    for q in range(n - 1):
        s_c = 1 << (n - 1 - q)
        s_t = 1 << (n - 2 - q)
        cd = (cols // s_c) % 2
        td = (cols // s_t) % 2
        rows = cols + (((cd + td) % 2) - td) * s_t
        Mr2 = np.empty_like(Mr)
        Mi2 = np.empty_like(Mi)
        Mr2[rows] = Mr
        Mi2[rows] = Mi
        Mr, Mi = Mr2, Mi2

    # CRX ladder: M'[i] = cos(a_i) M[i] - 1j sin(a_i) M[flip(i)]
    ang = np.asarray(angles, dtype=np.float64)
    for q in range(n - 1):
        s_c = 1 << (n - 1 - q)
        s_t = 1 << (n - 2 - q)
        cd = (cols // s_c) % 2
        td = (cols // s_t) % 2
        a = cd * (ang[q] / 2.0)
        flip = cols + (1 - 2 * td) * s_t
        ca = np.cos(a)[:, None]
        sa = np.sin(a)[:, None]
        Mr, Mi = ca * Mr + sa * Mi[flip], ca * Mi - sa * Mr[flip]

    return Mr.astype(np.float32), Mi.astype(np.float32)


def _get_program():
    if "nc" in _PROG:
        return _PROG["nc"]

    import concourse.bacc as bacc
    import concourse.mybir as mybir
    import concourse.tile as tile

    f32 = mybir.dt.float32
    KC = _D // 128      # 16 contraction chunks
    MC = _ROWS // 128   # 2 output-row chunks
    NB = 2 * _B         # 64: [x_re | x_im] columns

    # blob layout per partition: [ x (KC*NB) | wr (KC*_ROWS) | wi (KC*_ROWS) ]
    XW = KC * NB           # 1024
    WW = KC * _ROWS        # 4096
    BLOB = XW + 2 * WW     # 9216

    nc = bacc.Bacc("TRN2", target_bir_lowering=False)
    blob = nc.declare_dram_parameter("blob", [128, BLOB], f32, isOutput=False)
    # output in SBUF layout: o[p, mc*NB + n] = result row (mc*128+p), col n
    o = nc.declare_dram_parameter("o", [128, MC * NB], f32, isOutput=True)

    with tile.TileContext(nc) as tc:
        with (
            tc.tile_pool(name="bp", bufs=1) as bp,
            tc.tile_pool(name="ps", bufs=2, space="PSUM") as ps,
            tc.tile_pool(name="op", bufs=2) as op,
        ):
            b_sb = bp.tile([128, BLOB], f32)
            nc.sync.dma_start(b_sb[:], blob[:])
            x_sb = b_sb[:, 0:XW]
            wr_sb = b_sb[:, XW : XW + WW]
            wi_sb = b_sb[:, XW + WW : XW + 2 * WW]

            out_sb = op.tile([128, MC * NB], f32)
            for mc in range(MC):
                psum_r = ps.tile([128, NB], f32, tag="pr")
                for kc in range(KC):
                    nc.tensor.matmul(
                        psum_r[:],
                        wr_sb[:, kc * _ROWS + mc * 128 : kc * _ROWS + mc * 128 + 128],
                        x_sb[:, kc * NB : (kc + 1) * NB],
                        start=(kc == 0),
                        stop=(kc == KC - 1),
                    )
                psum_i = ps.tile([128, NB], f32, tag="pi")
                for kc in range(KC):
                    nc.tensor.matmul(
                        psum_i[:],
                        wi_sb[:, kc * _ROWS + mc * 128 : kc * _ROWS + mc * 128 + 128],
                        x_sb[:, kc * NB : (kc + 1) * NB],
                        start=(kc == 0),
                        stop=(kc == KC - 1),
                    )
                # o_re = Ur@x_re - Ui@x_im ; o_im = Ur@x_im + Ui@x_re
                # copy the *later* PE group first so each DVE op carries
                # at most one new sync wait (HW limit: 1 wait per instr).
                ob = out_sb[:, mc * NB : (mc + 1) * NB]
                nc.vector.tensor_copy(ob[:, 0:_B], psum_i[:, _B:NB])
                nc.vector.tensor_copy(ob[:, _B:NB], psum_i[:, 0:_B])
                nc.vector.tensor_sub(ob[:, 0:_B], psum_r[:, 0:_B], ob[:, 0:_B])
                nc.vector.tensor_add(ob[:, _B:NB], psum_r[:, _B:NB], ob[:, _B:NB])
            nc.sync.dma_start(o[:], out_sb[:])

    nc.compile()
    _PROG["nc"] = nc
    return nc


def kernel(x_real, x_imag, angles, dim, wires):
    from concourse.bass_utils import run_bass_kernel_spmd

    assert int(dim) == 2 and int(wires) == _N
    x_real = np.asarray(x_real, dtype=np.float32)
    x_imag = np.asarray(x_imag, dtype=np.float32)
    angles = np.asarray(angles, dtype=np.float32)

    Ur, Ui = _build_unitary(angles)
    xcat = np.concatenate([x_real, x_imag], axis=1)  # (D, 64)

    def _swz(a):
        # (2048, inner) -> SBUF layout (128, 16*inner): p, (kc, inner)
        inner = a.shape[1]
        return np.ascontiguousarray(
            a.reshape(16, 128, inner).transpose(1, 0, 2).reshape(128, 16 * inner)
        )

    x_swz = _swz(xcat)
    in_maps = []
    for c in range(_NC):
        sl = slice(c * _ROWS, (c + 1) * _ROWS)
        blob = np.concatenate(
            [
                x_swz,
                _swz(np.ascontiguousarray(Ur[sl, :].T)),
                _swz(np.ascontiguousarray(Ui[sl, :].T)),
            ],
            axis=1,
        )
        in_maps.append({"blob": np.ascontiguousarray(blob)})

    nc = _get_program()
    res = run_bass_kernel_spmd(nc, in_maps, list(range(_NC)))
    _LAST["res"] = res

    parts = []
    for c in range(_NC):
        od = res.results[c]["o"]  # (128, MC*64) SBUF layout
        parts.append(od.reshape(128, 2, 64).transpose(1, 0, 2).reshape(_ROWS, 64))
    o_all = np.concatenate(parts, axis=0)  # (2048, 64)
    return (o_all[:, :_B] + 1j * o_all[:, _B:]).astype(np.complex64)


# revision 14
# speedup vs baseline: 2.7005x; 2.7005x over previous
"""Trainium2 kernel for nn_Circuit_88871463288913.

Circuit (d=2, n=11 wires, D=2048, B=32):
  psi -> H on every wire -> CNOT ladder -> CRX ladder.

Decomposition (bits q0..q10, q10 fastest; major m = q0..q6, minor mu =
q7..q10):
  - H^11 = H7(major) x H4(minor)
  - CNOT ladder = prefix-XOR permutation = PX7(major), then minor map
    mu' = PX4(mu) ^ (q6' ? 1111 : 0)  conditioned on the post-PX7 bit q6'
  - CRX 0..5 act on major only (complex 128x128 product C)
  - CRX 6 = R_g on q7 conditioned on major bit q6' (g)
  - CRX 7..9 act on minor only (complex 16x16 product K)

Device pipeline per core (batch shard: 4 columns/core).  The state is one
128x128 f32 tile; every gate is a PE matmul with the STATE as the
stationary operand (lhsT), so each matmul applies the gate AND flips the
layout between (major | rho) and (rho | major), rho = reim*64 + mu*4 + b.
A bit-rotation relabel (Prot: m' -> m'' = q6'*64 + rest) makes the q6'
conditioning contiguous halves.

  MM1: rhs = (Prot P7 H7)^T                     -> (rho, m'')
  MM2: rhs = (F_h H4 (x) I4 (x) I2)^T, per half -> (m'', rho)
  MM3: complex C via PSUM accumulation with [Cre | -Cim] stacking
                                                -> (rho, m'')
  MM4: rhs = packed-complex (K R_g)^T, per half -> (m'', rho)

Everything angle-dependent is composed on the host in O(128^2).
"""

import numpy as np

_D = 2048
_N = 11
_B = 32
_NC = 8

_LAST = {}  # stash of the most recent BassKernelResults (for test harness)
_PROG = {}  # cached Bass program
_CONST = {}  # cached angle-independent matrices


def _walsh(nbits):
    n = 1 << nbits
    i = np.arange(n)
    v = i[:, None] & i[None, :]
    v = v ^ (v >> 8)
    v = v ^ (v >> 4)
    v = v ^ (v >> 2)
    v = v ^ (v >> 1)
    return (1.0 - 2.0 * (v & 1)) / np.sqrt(n)


def _prefix_xor(nbits):
    v = np.arange(1 << nbits)
    res = np.zeros_like(v)
    acc = np.zeros_like(v)
    for k in range(nbits):
        bit = (v >> (nbits - 1 - k)) & 1
        acc = acc ^ bit
        res = res | (acc << (nbits - 1 - k))
    return res


def _perm_mat(p):
    n = len(p)
    M = np.zeros((n, n))
    M[p, np.arange(n)] = 1.0
    return M


def _crx(nbits, cpos, tpos, theta):
    n = 1 << nbits
    i = np.arange(n)
    cd = (i >> (nbits - 1 - cpos)) & 1
    a = cd * theta / 2.0
    flip = i ^ (1 << (nbits - 1 - tpos))
    M = np.zeros((n, n), dtype=complex)
    M[i, i] = np.cos(a)
    M[flip, i] = -1j * np.sin(a)
    return M


def _expand_mu(Mmu):  # 16x16 -> 64x64 (tensor with I4 over batch)
    return np.kron(Mmu, np.eye(4))


def _get_const():
    if _CONST:
        return _CONST
    m = np.arange(128)
    rot = ((m & 1) << 6) | (m >> 1)  # m' -> m''
    inv = np.empty(128, dtype=int)
    inv[rot] = m
    Prot = _perm_mat(rot)
    M1 = Prot @ _perm_mat(_prefix_xor(7)) @ _walsh(7)
    PX4 = _prefix_xor(4)
    H4 = _walsh(4)
    G = [_perm_mat(PX4 ^ (15 * h)) @ H4 for h in (0, 1)]
    W1 = [np.kron(np.eye(2), _expand_mu(G[h])) for h in (0, 1)]
    _CONST.update(rot=rot, inv=inv, Prot=Prot, M1=M1, W1=W1)
    return _CONST


def _build_weights(angles):
    """Returns the 8 rhs matrices (f32, 128x128 each), already transposed."""
    cst = _get_const()
    ang = np.asarray(angles, dtype=np.float64)

    C = np.eye(128, dtype=complex)
    for q in range(6):
        C = _crx(7, q, q + 1, ang[q]) @ C
    Cpp = cst["Prot"] @ C @ cst["Prot"].T

    K = np.eye(16, dtype=complex)
    for q in range(7, 10):
        K = _crx(4, q - 7, q - 6, ang[q]) @ K
    W2 = []
    for g in (0, 1):
        i = np.arange(16)
        a = g * ang[6] / 2.0
        R = np.zeros((16, 16), dtype=complex)
        R[i, i] = np.cos(a)
        R[i ^ 8, i] = -1j * np.sin(a)
        Kg = K @ R
        Kr, Ki = _expand_mu(np.real(Kg)), _expand_mu(np.imag(Kg))
        W2.append(np.block([[Kr, -Ki], [Ki, Kr]]))

    Cre, Cim = np.real(Cpp), np.imag(Cpp)
    mats = [
        cst["M1"].T,
        cst["W1"][0].T,
        cst["W1"][1].T,
        Cre.T,
        Cim.T,
        (-Cim).T,
        W2[0].T,
        W2[1].T,
    ]
    return [np.ascontiguousarray(x, dtype=np.float32) for x in mats]


def _get_program():
    if "nc" in _PROG:
        return _PROG["nc"]

    import concourse.bacc as bacc
    import concourse.mybir as mybir
    import concourse.tile as tile

    f32 = mybir.dt.float32
    # blob cols: [A0 | M1T | W10T | W11T | CreT | CimT | nCimT | W20T | W21T]
    NW = 9
    BLOB = NW * 128

    nc = bacc.Bacc("TRN2", target_bir_lowering=False)
    blob = nc.declare_dram_parameter("blob", [128, BLOB], f32, isOutput=False)
    o = nc.declare_dram_parameter("o", [128, 128], f32, isOutput=True)

    with tile.TileContext(nc) as tc:
        with (
            tc.tile_pool(name="bp", bufs=1) as bp,
            tc.tile_pool(name="ps", bufs=4, space="PSUM") as ps,
            tc.tile_pool(name="sp", bufs=4) as sp,
        ):
            b_sb = bp.tile([128, BLOB], f32)
            nc.sync.dma_start(b_sb[:], blob[:])
            sl = lambda k: b_sb[:, k * 128 : (k + 1) * 128]
            A0, M1T, W10T, W11T, CreT, CimT, nCimT, W20T, W21T = (
                sl(k) for k in range(9)
            )

            p1 = ps.tile([128, 128], f32, tag="p")
            nc.tensor.matmul(p1[:], A0, M1T, start=True, stop=True)
            s1 = sp.tile([128, 128], f32, tag="s")
            nc.vector.tensor_copy(s1[:], p1[:])

            p2 = ps.tile([128, 128], f32, tag="p")
            nc.tensor.matmul(p2[0:64, :], s1[:, 0:64], W10T, start=True, stop=True)
            nc.tensor.matmul(p2[64:128, :], s1[:, 64:128], W11T, start=True, stop=True)
            s2 = sp.tile([128, 128], f32, tag="s")
            nc.vector.tensor_copy(s2[:], p2[:])

            p3 = ps.tile([128, 128], f32, tag="p")
            nc.tensor.matmul(p3[0:64, :], s2[:, 0:64], CreT, start=True, stop=False)
            nc.tensor.matmul(p3[0:64, :], s2[:, 64:128], nCimT, start=False, stop=True)
            nc.tensor.matmul(p3[64:128, :], s2[:, 0:64], CimT, start=True, stop=False)
            nc.tensor.matmul(p3[64:128, :], s2[:, 64:128], CreT, start=False, stop=True)
            s3 = sp.tile([128, 128], f32, tag="s")
            nc.vector.tensor_copy(s3[:], p3[:])

            p4 = ps.tile([128, 128], f32, tag="p")
            nc.tensor.matmul(p4[0:64, :], s3[:, 0:64], W20T, start=True, stop=True)
            nc.tensor.matmul(p4[64:128, :], s3[:, 64:128], W21T, start=True, stop=True)
            s4 = sp.tile([128, 128], f32, tag="s")
            nc.vector.tensor_copy(s4[:], p4[:])

            nc.sync.dma_start(o[:], s4[:])

    nc.compile()
    _PROG["nc"] = nc
    return nc


def kernel(x_real, x_imag, angles, dim, wires):
    from concourse.bass_utils import run_bass_kernel_spmd

    assert int(dim) == 2 and int(wires) == _N
    x_real = np.asarray(x_real, dtype=np.float32)
    x_imag = np.asarray(x_imag, dtype=np.float32)

    W = np.concatenate(_build_weights(angles), axis=1)  # (128, 8*128)

    in_maps = []
    for c in range(_NC):
        cols = slice(4 * c, 4 * c + 4)
        # A0[m, reim*64 + mu*4 + b] = x[m*16+mu, 4c+b]
        a_re = x_real[:, cols].reshape(128, 64)
        a_im = x_imag[:, cols].reshape(128, 64)
        blob = np.concatenate([a_re, a_im, W], axis=1)
        in_maps.append({"blob": np.ascontiguousarray(blob)})

    nc = _get_program()
    res = run_bass_kernel_spmd(nc, in_maps, list(range(_NC)))
    _LAST["res"] = res

    inv = _get_const()["inv"]
    out = np.empty((_D, _B), dtype=np.complex64)
    for c in range(_NC):
        od = res.results[c]["o"]  # (128=m'', 128=rho)
        z = od[:, 0:64].reshape(128, 16, 4) + 1j * od[:, 64:128].reshape(128, 16, 4)
        full = np.empty((128, 16, 4), dtype=np.complex64)
        full[inv] = z  # basis m' = inv[m'']
        out[:, 4 * c : 4 * c + 4] = full.reshape(_D, 4)
    return out


# revision 16
# speedup vs baseline: 3.1017x; 1.1486x over previous
"""Trainium2 kernel for nn_Circuit_88871463288913.

Circuit (d=2, n=11 wires, D=2048, B=32):
  psi -> H on every wire -> CNOT ladder -> CRX ladder.

Decomposition (bits q0..q10, q10 fastest; major m = q0..q6, minor mu =
q7..q10):
  - H^11 = H7(major) x H4(minor)
  - CNOT ladder = prefix-XOR permutation = PX7(major), then minor map
    mu' = PX4(mu) ^ (q6' ? 1111 : 0)  conditioned on the post-PX7 bit q6'
  - CRX 0..5 act on major only (complex 128x128 product C)
  - CRX 6 = R_g on q7 conditioned on major bit q6' (g)
  - CRX 7..9 act on minor only (complex 16x16 product K)

Device pipeline per core (batch shard: 4 columns/core).  The state is one
128x128 f32 tile; every gate is a PE matmul with the STATE as the
stationary operand (lhsT), so each matmul applies the gate AND flips the
layout between (major | rho) and (rho | major), rho = reim*64 + mu*4 + b.
A bit-rotation relabel (Prot: m' -> m'' = q6'*64 + rest) makes the q6'
conditioning contiguous halves.

  MM1: rhs = (Prot P7 H7)^T                     -> (rho, m'')
  MM2: rhs = (F_h H4 (x) I4 (x) I2)^T, per half -> (m'', rho)
  MM3: complex C via PSUM accumulation with [Cre | -Cim] stacking
                                                -> (rho, m'')
  MM4: rhs = packed-complex (K R_g)^T, per half -> (m'', rho)

Everything angle-dependent is composed on the host in O(128^2).
"""

import numpy as np

_D = 2048
_N = 11
_B = 32
_NC = 8

_LAST = {}  # stash of the most recent BassKernelResults (for test harness)
_PROG = {}  # cached Bass program
_CONST = {}  # cached angle-independent matrices
_FP16 = True  # device dtype for state/weights (PSUM accumulate is always f32)


def _walsh(nbits):
    n = 1 << nbits
    i = np.arange(n)
    v = i[:, None] & i[None, :]
    v = v ^ (v >> 8)
    v = v ^ (v >> 4)
    v = v ^ (v >> 2)
    v = v ^ (v >> 1)
    return (1.0 - 2.0 * (v & 1)) / np.sqrt(n)


def _prefix_xor(nbits):
    v = np.arange(1 << nbits)
    res = np.zeros_like(v)
    acc = np.zeros_like(v)
    for k in range(nbits):
        bit = (v >> (nbits - 1 - k)) & 1
        acc = acc ^ bit
        res = res | (acc << (nbits - 1 - k))
    return res


def _perm_mat(p):
    n = len(p)
    M = np.zeros((n, n))
    M[p, np.arange(n)] = 1.0
    return M


def _crx(nbits, cpos, tpos, theta):
    n = 1 << nbits
    i = np.arange(n)
    cd = (i >> (nbits - 1 - cpos)) & 1
    a = cd * theta / 2.0
    flip = i ^ (1 << (nbits - 1 - tpos))
    M = np.zeros((n, n), dtype=complex)
    M[i, i] = np.cos(a)
    M[flip, i] = -1j * np.sin(a)
    return M


def _expand_mu(Mmu):  # 16x16 -> 64x64 (tensor with I4 over batch)
    return np.kron(Mmu, np.eye(4))


def _get_const():
    if _CONST:
        return _CONST
    m = np.arange(128)
    rot = ((m & 1) << 6) | (m >> 1)  # m' -> m''
    inv = np.empty(128, dtype=int)
    inv[rot] = m
    Prot = _perm_mat(rot)
    M1 = Prot @ _perm_mat(_prefix_xor(7)) @ _walsh(7)
    PX4 = _prefix_xor(4)
    H4 = _walsh(4)
    G = [_perm_mat(PX4 ^ (15 * h)) @ H4 for h in (0, 1)]
    W1 = [np.kron(np.eye(2), _expand_mu(G[h])) for h in (0, 1)]
    _CONST.update(rot=rot, inv=inv, Prot=Prot, M1=M1, W1=W1)
    return _CONST


def _build_weights(angles):
    """Returns the 8 rhs matrices (f32, 128x128 each), already transposed."""
    cst = _get_const()
    ang = np.asarray(angles, dtype=np.float64)

    C = np.eye(128, dtype=complex)
    for q in range(6):
        C = _crx(7, q, q + 1, ang[q]) @ C
    Cpp = cst["Prot"] @ C @ cst["Prot"].T

    K = np.eye(16, dtype=complex)
    for q in range(7, 10):
        K = _crx(4, q - 7, q - 6, ang[q]) @ K
    W2 = []
    for g in (0, 1):
        i = np.arange(16)
        a = g * ang[6] / 2.0
        R = np.zeros((16, 16), dtype=complex)
        R[i, i] = np.cos(a)
        R[i ^ 8, i] = -1j * np.sin(a)
        Kg = K @ R
        Kr, Ki = _expand_mu(np.real(Kg)), _expand_mu(np.imag(Kg))
        W2.append(np.block([[Kr, -Ki], [Ki, Kr]]))

    Cre, Cim = np.real(Cpp), np.imag(Cpp)
    mats = [
        cst["M1"].T,
        cst["W1"][0].T,
        cst["W1"][1].T,
        Cre.T,
        Cim.T,
        (-Cim).T,
        W2[0].T,
        W2[1].T,
    ]
    return [np.ascontiguousarray(x, dtype=np.float32) for x in mats]


def _get_program():
    if "nc" in _PROG:
        return _PROG["nc"]

    import concourse.bacc as bacc
    import concourse.mybir as mybir
    import concourse.tile as tile

    f32 = mybir.dt.float32
    dt = mybir.dt.float16 if _FP16 else mybir.dt.float32
    # blob cols: [A0 | M1T | W10T | W11T | CreT | CimT | nCimT | W20T | W21T]
    NW = 9
    BLOB = NW * 128

    nc = bacc.Bacc("TRN2", target_bir_lowering=False)
    blob = nc.declare_dram_parameter("blob", [128, BLOB], dt, isOutput=False)
    o = nc.declare_dram_parameter("o", [128, 128], f32, isOutput=True)

    with tile.TileContext(nc) as tc:
        with (
            tc.tile_pool(name="bp", bufs=1) as bp,
            tc.tile_pool(name="ps", bufs=4, space="PSUM") as ps,
            tc.tile_pool(name="sp", bufs=4) as sp,
        ):
            b_sb = bp.tile([128, BLOB], dt)
            nc.sync.dma_start(b_sb[:], blob[:])
            sl = lambda k: b_sb[:, k * 128 : (k + 1) * 128]
            A0, M1T, W10T, W11T, CreT, CimT, nCimT, W20T, W21T = (
                sl(k) for k in range(9)
            )

            p1 = ps.tile([128, 128], f32, tag="p")
            nc.tensor.matmul(p1[:], A0, M1T, start=True, stop=True)
            s1 = sp.tile([128, 128], dt, tag="s")
            nc.vector.tensor_copy(s1[:], p1[:])

            p2 = ps.tile([128, 128], f32, tag="p")
            nc.tensor.matmul(p2[0:64, :], s1[:, 0:64], W10T, start=True, stop=True)
            nc.tensor.matmul(p2[64:128, :], s1[:, 64:128], W11T, start=True, stop=True)
            s2 = sp.tile([128, 128], dt, tag="s")
            nc.vector.tensor_copy(s2[:], p2[:])

            p3 = ps.tile([128, 128], f32, tag="p")
            nc.tensor.matmul(p3[0:64, :], s2[:, 0:64], CreT, start=True, stop=False)
            nc.tensor.matmul(p3[0:64, :], s2[:, 64:128], nCimT, start=False, stop=True)
            nc.tensor.matmul(p3[64:128, :], s2[:, 0:64], CimT, start=True, stop=False)
            nc.tensor.matmul(p3[64:128, :], s2[:, 64:128], CreT, start=False, stop=True)
            s3 = sp.tile([128, 128], dt, tag="s")
            nc.vector.tensor_copy(s3[:], p3[:])

            p4 = ps.tile([128, 128], f32, tag="p")
            nc.tensor.matmul(p4[0:64, :], s3[:, 0:64], W20T, start=True, stop=True)
            nc.tensor.matmul(p4[64:128, :], s3[:, 64:128], W21T, start=True, stop=True)
            s4 = sp.tile([128, 128], f32, tag="s")
            nc.vector.tensor_copy(s4[:], p4[:])

            nc.sync.dma_start(o[:], s4[:])

    nc.compile()
    _PROG["nc"] = nc
    return nc


def kernel(x_real, x_imag, angles, dim, wires):
    from concourse.bass_utils import run_bass_kernel_spmd

    assert int(dim) == 2 and int(wires) == _N
    x_real = np.asarray(x_real, dtype=np.float32)
    x_imag = np.asarray(x_imag, dtype=np.float32)

    W = np.concatenate(_build_weights(angles), axis=1)  # (128, 8*128)

    in_maps = []
    for c in range(_NC):
        cols = slice(4 * c, 4 * c + 4)
        # A0[m, reim*64 + mu*4 + b] = x[m*16+mu, 4c+b]
        a_re = x_real[:, cols].reshape(128, 64)
        a_im = x_imag[:, cols].reshape(128, 64)
        blob = np.concatenate([a_re, a_im, W], axis=1)
        if _FP16:
            blob = blob.astype(np.float16)
        in_maps.append({"blob": np.ascontiguousarray(blob)})

    nc = _get_program()
    res = run_bass_kernel_spmd(nc, in_maps, list(range(_NC)))
    _LAST["res"] = res

    inv = _get_const()["inv"]
    out = np.empty((_D, _B), dtype=np.complex64)
    for c in range(_NC):
        od = res.results[c]["o"]  # (128=m'', 128=rho)
        z = od[:, 0:64].reshape(128, 16, 4) + 1j * od[:, 64:128].reshape(128, 16, 4)
        full = np.empty((128, 16, 4), dtype=np.complex64)
        full[inv] = z  # basis m' = inv[m'']
        out[:, 4 * c : 4 * c + 4] = full.reshape(_D, 4)
    return out
